# revision 23
# baseline (speedup 1.0000x reference)
"""Single-head attention with additive relative-position bias, data-parallel
over batch across 8 TRN2 NeuronCores.

Reference computation (per batch b):
    q = x @ Wq.T; k = x @ Wk.T; v = x @ Wv.T          # [S, D]
    scores = q @ k.T / sqrt(D) + bias                 # bias = emb[rel_pos]
    out = softmax(scores, -1) @ v

Device strategy (per core = one batch):
  * all PE operands bf16, PSUM accumulation f32
  * scores computed TRANSPOSED (S^T[ks, qs]) so that the softmax weights come
    out of the PE already in the [ks (partition), qs (free)] layout the
    attention@V matmul needs as its stationary operand -> no transposes at all.
  * row sums (softmax denominators) via matmul with a ones vector; the
    normalization is applied to the output block (per-partition scale).
  * exp() has no max-subtraction: logits are ~N(0,1) for these inputs
    (|logit| < ~8), safely inside f32/exp range.
  * 1/sqrt(D) is folded into Wq on the host.

Host-side prep is layout only: transposes/casts of inputs and the
emb[rel_pos] table lookup that produces the bias matrix.
"""

import numpy as np
import ml_dtypes

import concourse.bass as bass
import concourse.mybir as mybir
from concourse import bacc
from concourse import bass_utils as _bass_utils
from concourse.tile import TileContext
from concourse.bass_utils import run_bass_kernel_spmd

def _dedup_ldweights(nc) -> int:
    """Remove InstLdweights that reload the exact weights already in the PE
    array. The Tile lowering emits one LDWEIGHTS per matmul; on silicon each
    weight swap costs ~46ns of PE time (array drain before the next fill), so
    back-to-back matmuls sharing a stationary should load it once. Only
    sync-free LDWs are removed: any cross-engine hazard on the weights tile
    would surface as an on_wait on the LDW, which keeps it.
    """

    def sig(inst):
        ap = inst.ins[0]
        return (ap.memref, ap.offset, str(ap.ap), str(ap.dtype))

    removed = 0
    for blk in nc.m.functions[0].blocks:
        last_sig = None
        keep = []
        for inst in blk.instructions:
            tn = type(inst).__name__
            if str(getattr(inst, "engine", "")) == "EngineType.PE":
                if tn == "InstLdweights":
                    si = inst.sync_info
                    clean = si is None or (not si.on_wait and not si.on_update)
                    if clean and last_sig == sig(inst):
                        removed += 1
                        continue  # drop: same weights already loaded
                    last_sig = sig(inst)
                elif tn != "InstMatmult":
                    last_sig = None  # drains/branches etc: be conservative
            keep.append(inst)
        if removed:
            blk.instructions[:] = keep
    return removed

BF16 = mybir.dt.bfloat16
F32 = mybir.dt.float32
BF16_NP = ml_dtypes.bfloat16

B = 8
N_CORES = 8
P = 128  # partitions


def build_attention_nc(S: int, D: int) -> bass.Bass:
    """Build the single-core graph (SPMD: same graph on all 8 cores)."""
    assert S % 512 == 0 and D % 512 == 0
    FT = D // P          # contraction tiles over d_in
    OT = D // P          # tiles over d_out
    ST = S // P          # seq tiles of 128
    NPANEL = S // 512    # qs panels of 512
    KST = S // P         # ks tiles of 128
    DH = D // 512        # 512-wide halves of d_out

    nc = bacc.Bacc(None, target_bir_lowering=False)

    xT_d = nc.declare_dram_parameter("xT", [D, S], BF16, isOutput=False)
    wqT_d = nc.declare_dram_parameter("wqT", [D, D], BF16, isOutput=False)
    wkT_d = nc.declare_dram_parameter("wkT", [D, D], BF16, isOutput=False)
    wvT_d = nc.declare_dram_parameter("wvT", [D, D], BF16, isOutput=False)
    biasT_d = nc.declare_dram_parameter("biasT", [S, S], BF16, isOutput=False)
    out_d = nc.declare_dram_parameter("out", [S, D], F32, isOutput=True)

    with TileContext(nc) as tc:
        # ---- persistent activations (live across both phases) ----
        with (
            tc.tile_pool(name="persist", bufs=1) as persist,
            tc.tile_pool(name="small", bufs=1) as small,
        ):
            QT = [persist.tile([P, S], BF16, name=f"qt{i}") for i in range(OT)]
            KT = [persist.tile([P, S], BF16, name=f"kt{i}") for i in range(OT)]
            V = [persist.tile([P, D], BF16, name=f"v{i}") for i in range(ST)]
            ones = small.tile([P, 1], BF16, name="ones")
            nc.vector.memset(ones, 1.0)

            # ================= Phase A: projections =================
            with (
                tc.tile_pool(name="xw", bufs=1) as xw,
                tc.tile_pool(name="psA", bufs=2, space="PSUM") as psA,
            ):
                XT = [xw.tile([P, S], BF16, name=f"xt{i}") for i in range(FT)]
                WQ = [xw.tile([P, D], BF16, name=f"wq{i}") for i in range(FT)]
                WK = [xw.tile([P, D], BF16, name=f"wk{i}") for i in range(FT)]
                WV = [xw.tile([P, D], BF16, name=f"wv{i}") for i in range(FT)]
                # Startup critical path: the V projection runs first and its
                # st-th group consumes only columns [st*128,(st+1)*128) of each
                # XT tile, so compute can start after WV + the first XT slices
                # land; the rest of XT/WQ/WK stream in under the V sweep.
                SW = min(1024, S)
                for i in range(FT):
                    nc.sync.dma_start(out=WV[i], in_=wvT_d[i * P:(i + 1) * P, :])
                for i in range(FT):
                    nc.sync.dma_start(out=XT[i][:, 0:SW],
                                      in_=xT_d[i * P:(i + 1) * P, 0:SW])
                for i in range(FT):
                    if SW < S:
                        nc.sync.dma_start(out=XT[i][:, SW:],
                                          in_=xT_d[i * P:(i + 1) * P, SW:])
                for i in range(FT):
                    nc.sync.dma_start(out=WQ[i], in_=wqT_d[i * P:(i + 1) * P, :])
                for i in range(FT):
                    nc.sync.dma_start(out=WK[i], in_=wkT_d[i * P:(i + 1) * P, :])

                # V: [s (part), o (free)] = x.T.T @ Wv.T
                DW = min(1024, D)
                for st in range(ST):
                    for oh in range(D // DW):
                        ps = psA.tile([P, DW], F32, name="psA")
                        for ft in range(FT):
                            for half in range(DW // 512):
                                nc.tensor.matmul(
                                    ps[:, half * 512:(half + 1) * 512],
                                    lhsT=XT[ft][:, st * P:(st + 1) * P],
                                    rhs=WV[ft][:, oh * DW + half * 512:
                                               oh * DW + (half + 1) * 512],
                                    start=(ft == 0),
                                    stop=(ft == FT - 1),
                                )
                        nc.scalar.activation(
                            V[st][:, oh * DW:(oh + 1) * DW], ps,
                            mybir.ActivationFunctionType.Copy,
                        )

                # Q^T and K^T: [o (part), s (free)] = W.T.T @ x.T
                # Full-S psum strip: each stationary W tile feeds S//512
                # back-to-back matmuls (one LDWEIGHTS after dedup).
                for W_sb, dst in ((WQ, QT), (WK, KT)):
                    for ot in range(OT):
                        ps = psA.tile([P, S], F32, name="psA")
                        for ft in range(FT):
                            for half in range(S // 512):
                                nc.tensor.matmul(
                                    ps[:, half * 512:(half + 1) * 512],
                                    lhsT=W_sb[ft][:, ot * P:(ot + 1) * P],
                                    rhs=XT[ft][:, half * 512:(half + 1) * 512],
                                    start=(ft == 0),
                                    stop=(ft == FT - 1),
                                )
                        for half in range(S // SW):
                            nc.scalar.activation(
                                dst[ot][:, half * SW:(half + 1) * SW],
                                ps[:, half * SW:(half + 1) * SW],
                                mybir.ActivationFunctionType.Copy,
                            )

            # ================= Phase B: attention =================
            # pass 1: the full expS^T matrix [S, S] in SBUF (bf16), computed
            # ks-tile-major with the panel loop innermost so each stationary
            # KT slice feeds NPANEL back-to-back matmuls (one LDWEIGHTS).
            es = {}
            with tc.tile_pool(name="es", bufs=NPANEL * KST) as es_pool:
              with (
                tc.tile_pool(name="bt", bufs=6) as bt_pool,
                tc.tile_pool(name="stg", bufs=6) as stg_pool,
                tc.tile_pool(name="psS", bufs=2 * NPANEL, space="PSUM") as psS,
              ):
                for kt in range(KST):
                    pss = [psS.tile([P, 512], F32, name="psS")
                           for _ in range(NPANEL)]
                    for ot in range(OT):
                        for panel in range(NPANEL):
                            nc.tensor.matmul(
                                pss[panel],
                                lhsT=KT[ot][:, kt * P:(kt + 1) * P],
                                rhs=QT[ot][:, panel * 512:(panel + 1) * 512],
                                start=(ot == 0),
                                stop=(ot == OT - 1),
                            )
                    for panel in range(NPANEL):
                        bt = bt_pool.tile([P, 512], BF16, name="bt")
                        nc.sync.dma_start(
                            out=bt, in_=biasT_d[kt * P:(kt + 1) * P,
                                               panel * 512:(panel + 1) * 512])
                        stg = stg_pool.tile([P, 512], F32, name="stg")
                        nc.vector.tensor_add(stg, pss[panel], bt)
                        e = es_pool.tile([P, 512], BF16, name="es")
                        nc.scalar.activation(
                            e, stg, mybir.ActivationFunctionType.Exp)
                        es[(kt, panel)] = e

              # pass 2: O[qs, :] = softmax @ V, one 128-row block at a time;
              # each stationary expS^T slice feeds 3 matmuls (V halves, ones).
              with (
                tc.tile_pool(name="ob", bufs=3) as ob_pool,
                tc.tile_pool(name="rc", bufs=4) as rc_pool,
                tc.tile_pool(name="psO", bufs=3, space="PSUM") as psO,
                tc.tile_pool(name="psD", bufs=2, space="PSUM") as psD,
              ):
                for panel in range(NPANEL):
                    q0 = panel * 512
                    for j in range(4):
                        po = psO.tile([P, D], F32, name="psO")
                        pd = psD.tile([P, 1], F32, name="psD")
                        for kt in range(KST):
                            w_sb = es[(kt, panel)][:, j * P:(j + 1) * P]
                            for half in range(DH):
                                nc.tensor.matmul(
                                    po[:, half * 512:(half + 1) * 512],
                                    lhsT=w_sb,
                                    rhs=V[kt][:, half * 512:(half + 1) * 512],
                                    start=(kt == 0),
                                    stop=(kt == KST - 1),
                                )
                            nc.tensor.matmul(
                                pd, lhsT=w_sb, rhs=ones,
                                start=(kt == 0), stop=(kt == KST - 1),
                            )
                        rec = rc_pool.tile([P, 1], F32, name="rc")
                        nc.vector.reciprocal(rec, pd)
                        ob = ob_pool.tile([P, D], F32, name="ob")
                        row = q0 + j * P
                        for half in range(2):
                            hs = slice(half * D // 2, (half + 1) * D // 2)
                            nc.scalar.activation(
                                ob[:, hs], po[:, hs],
                                mybir.ActivationFunctionType.Copy,
                                scale=rec[:, 0:1],
                            )
                            nc.sync.dma_start(
                                out=out_d[row:row + P, hs], in_=ob[:, hs])

    _dedup_ldweights(nc)
    nc.compile()
    return nc


_NC_CACHE: dict = {}


def _get_nc(S: int, D: int) -> bass.Bass:
    key = (S, D)
    if key not in _NC_CACHE:
        _NC_CACHE[key] = build_attention_nc(S, D)
    return _NC_CACHE[key]


def kernel(x, Wq, Wk, Wv, rel_pos_emb, rel_pos) -> np.ndarray:
    x = np.asarray(x, dtype=np.float32)
    Wq = np.asarray(Wq, dtype=np.float32)
    Wk = np.asarray(Wk, dtype=np.float32)
    Wv = np.asarray(Wv, dtype=np.float32)
    rel_pos_emb = np.asarray(rel_pos_emb, dtype=np.float32)
    rel_pos = np.asarray(rel_pos)

    b, S, D = x.shape
    assert b == B

    # host prep: layout transforms + bias table lookup
    scale = 1.0 / np.sqrt(np.float32(D))
    wqT = np.ascontiguousarray((Wq.T * scale)).astype(BF16_NP)
    wkT = np.ascontiguousarray(Wk.T).astype(BF16_NP)
    wvT = np.ascontiguousarray(Wv.T).astype(BF16_NP)
    bias = rel_pos_emb[rel_pos[:S, :S], 0]          # [qs, ks]
    biasT = np.ascontiguousarray(bias.T).astype(BF16_NP)  # [ks, qs]

    in_maps = []
    for i in range(N_CORES):
        in_maps.append({
            "xT": np.ascontiguousarray(x[i].T).astype(BF16_NP),
            "wqT": wqT,
            "wkT": wkT,
            "wvT": wvT,
            "biasT": biasT,
        })

    nc = _get_nc(S, D)
    res = run_bass_kernel_spmd(
        nc, in_maps, core_ids=list(range(N_CORES)), **_RUN_KWARGS)
    global LAST_RESULT
    LAST_RESULT = res
    return np.stack([r["out"] for r in res.results]).astype(np.float32)


# test harness hooks: set _RUN_KWARGS = {"trace": True} before calling kernel()
# to capture the NTFF profile; the full BassKernelResults lands in LAST_RESULT.
_RUN_KWARGS: dict = {}
LAST_RESULT = None


# revision 24
# speedup vs baseline: 1.1853x; 1.1853x over previous
"""Single-head attention with additive relative-position bias, data-parallel
over batch across 8 TRN2 NeuronCores.

Reference computation (per batch b):
    q = x @ Wq.T; k = x @ Wk.T; v = x @ Wv.T          # [S, D]
    scores = q @ k.T / sqrt(D) + bias                 # bias = emb[rel_pos]
    out = softmax(scores, -1) @ v

Device strategy (per core = one batch):
  * all PE operands bf16, PSUM accumulation f32
  * scores computed TRANSPOSED (S^T[ks, qs]) so that the softmax weights come
    out of the PE already in the [ks (partition), qs (free)] layout the
    attention@V matmul needs as its stationary operand -> no transposes at all.
  * row sums (softmax denominators) via matmul with a ones vector; the
    normalization is applied to the output block (per-partition scale).
  * exp() has no max-subtraction: logits are ~N(0,1) for these inputs
    (|logit| < ~8), safely inside f32/exp range.
  * 1/sqrt(D) is folded into Wq on the host.

Host-side prep is layout only: transposes/casts of inputs and the
emb[rel_pos] table lookup that produces the bias matrix.
"""

import numpy as np
import ml_dtypes

import concourse.bass as bass
import concourse.mybir as mybir
from concourse import bacc
from concourse import bass_utils as _bass_utils
from concourse.tile import TileContext
from concourse.bass_utils import run_bass_kernel_spmd

def _dedup_ldweights(nc) -> int:
    """Remove InstLdweights that reload the exact weights already in the PE
    array. The Tile lowering emits one LDWEIGHTS per matmul; on silicon each
    weight swap costs ~46ns of PE time (array drain before the next fill), so
    back-to-back matmuls sharing a stationary should load it once. Only
    sync-free LDWs are removed: any cross-engine hazard on the weights tile
    would surface as an on_wait on the LDW, which keeps it.
    """

    def sig(inst):
        ap = inst.ins[0]
        return (ap.memref, ap.offset, str(ap.ap), str(ap.dtype))

    removed = 0
    for blk in nc.m.functions[0].blocks:
        last_sig = None
        keep = []
        for inst in blk.instructions:
            tn = type(inst).__name__
            if str(getattr(inst, "engine", "")) == "EngineType.PE":
                if tn == "InstLdweights":
                    si = inst.sync_info
                    clean = si is None or (not si.on_wait and not si.on_update)
                    if clean and last_sig == sig(inst):
                        removed += 1
                        continue  # drop: same weights already loaded
                    last_sig = sig(inst)
                elif tn != "InstMatmult":
                    last_sig = None  # drains/branches etc: be conservative
            keep.append(inst)
        if removed:
            blk.instructions[:] = keep
    return removed

BF16 = mybir.dt.bfloat16
F32 = mybir.dt.float32
BF16_NP = ml_dtypes.bfloat16

B = 8
N_CORES = 8
P = 128  # partitions


def build_attention_nc(S: int, D: int) -> bass.Bass:
    """Build the single-core graph (SPMD: same graph on all 8 cores)."""
    assert S % 512 == 0 and D % 512 == 0
    FT = D // P          # contraction tiles over d_in
    OT = D // P          # tiles over d_out
    ST = S // P          # seq tiles of 128
    NPANEL = S // 512    # qs panels of 512
    KST = S // P         # ks tiles of 128
    DH = D // 512        # 512-wide halves of d_out
    SW = min(1024, S)
    DW = min(1024, D)

    nc = bacc.Bacc(None, target_bir_lowering=False)

    xT_d = nc.declare_dram_parameter("xT", [D, S], BF16, isOutput=False)
    wqT_d = nc.declare_dram_parameter("wqT", [D, D], BF16, isOutput=False)
    wkT_d = nc.declare_dram_parameter("wkT", [D, D], BF16, isOutput=False)
    wvT_d = nc.declare_dram_parameter("wvT", [D, D], BF16, isOutput=False)
    biasT_d = nc.declare_dram_parameter("biasT", [S, S], BF16, isOutput=False)
    out_d = nc.declare_dram_parameter("out", [S, D], F32, isOutput=True)

    with TileContext(nc) as tc:
        # ---- persistent activations (live across both phases) ----
        with (
            tc.tile_pool(name="persist", bufs=1) as persist,
            tc.tile_pool(name="small", bufs=1) as small,
        ):
            QT = [persist.tile([P, S], BF16, name=f"qt{i}") for i in range(OT)]
            KT = [persist.tile([P, S], BF16, name=f"kt{i}") for i in range(OT)]
            V = [persist.tile([P, D], BF16, name=f"v{i}") for i in range(ST)]
            ones = small.tile([P, 1], BF16, name="ones")
            nc.vector.memset(ones, 1.0)

            # ================= Phase A: projections =================
            with (
                tc.tile_pool(name="xw", bufs=1) as xw,
                tc.tile_pool(name="psA", bufs=3, space="PSUM") as psA,
            ):
                XT = [xw.tile([P, S], BF16, name=f"xt{i}") for i in range(FT)]
                WQ = [xw.tile([P, D], BF16, name=f"wq{i}") for i in range(FT)]
                WK = [xw.tile([P, D], BF16, name=f"wk{i}") for i in range(FT)]
                WV = [xw.tile([P, D], BF16, name=f"wv{i}") for i in range(FT)]
                # Startup critical path: the V projection runs first; its
                # st=0 group is split into o-halves so the first matmuls need
                # only XT[:, 0:128] slices + the first halves of WV. The rest
                # of XT/WQ/WK stream in under the V sweep.
                for i in range(FT):
                    nc.sync.dma_start(out=XT[i][:, 0:P],
                                      in_=xT_d[i * P:(i + 1) * P, 0:P])
                for half in range(DW // 512):
                    for i in range(FT):
                        hs = slice(half * 512, (half + 1) * 512)
                        nc.sync.dma_start(out=WV[i][:, hs],
                                          in_=wvT_d[i * P:(i + 1) * P, hs])
                for i in range(FT):
                    nc.sync.dma_start(out=XT[i][:, P:SW],
                                      in_=xT_d[i * P:(i + 1) * P, P:SW])
                for i in range(FT):
                    if SW < S:
                        nc.sync.dma_start(out=XT[i][:, SW:],
                                          in_=xT_d[i * P:(i + 1) * P, SW:])
                for i in range(FT):
                    nc.sync.dma_start(out=WQ[i], in_=wqT_d[i * P:(i + 1) * P, :])
                for i in range(FT):
                    nc.sync.dma_start(out=WK[i], in_=wkT_d[i * P:(i + 1) * P, :])

                # V: [s (part), o (free)] = x.T.T @ Wv.T
                for st in range(ST):
                    ohalves = DH if st == 0 else D // DW
                    width = 512 if st == 0 else DW
                    for oh in range(ohalves):
                        ps = psA.tile([P, width], F32, name="psA")
                        for ft in range(FT):
                            for half in range(width // 512):
                                o0 = oh * width + half * 512
                                nc.tensor.matmul(
                                    ps[:, half * 512:(half + 1) * 512],
                                    lhsT=XT[ft][:, st * P:(st + 1) * P],
                                    rhs=WV[ft][:, o0:o0 + 512],
                                    start=(ft == 0),
                                    stop=(ft == FT - 1),
                                )
                        nc.scalar.activation(
                            V[st][:, oh * width:(oh + 1) * width], ps,
                            mybir.ActivationFunctionType.Copy,
                        )

                # Q^T and K^T: [o (part), s (free)] = W.T.T @ x.T
                for W_sb, dst in ((WQ, QT), (WK, KT)):
                    for ot in range(OT):
                        for sh in range(S // SW):
                            ps = psA.tile([P, SW], F32, name="psA")
                            for ft in range(FT):
                                for half in range(SW // 512):
                                    nc.tensor.matmul(
                                        ps[:, half * 512:(half + 1) * 512],
                                        lhsT=W_sb[ft][:, ot * P:(ot + 1) * P],
                                        rhs=XT[ft][:, sh * SW + half * 512:
                                                   sh * SW + (half + 1) * 512],
                                        start=(ft == 0),
                                        stop=(ft == FT - 1),
                                    )
                            nc.scalar.activation(
                                dst[ot][:, sh * SW:(sh + 1) * SW], ps,
                                mybir.ActivationFunctionType.Copy,
                            )

            # ================= Phase B: attention =================
            # Per qs-panel: pass 1 computes the expS^T strip [ks, panel]
            # (scores transposed; bias added on DVE; exp on ACT -> bf16);
            # pass 2 multiplies the strip against V with the softmax weights
            # as the stationary operand, denominators via a ones matmul.
            with (
                tc.tile_pool(name="es", bufs=2 * KST) as es_pool,
                tc.tile_pool(name="bt", bufs=4) as bt_pool,
                tc.tile_pool(name="stg", bufs=4) as stg_pool,
                tc.tile_pool(name="ob", bufs=3) as ob_pool,
                tc.tile_pool(name="rc", bufs=4) as rc_pool,
                tc.tile_pool(name="psS", bufs=2, space="PSUM") as psS,
                tc.tile_pool(name="psO", bufs=2, space="PSUM") as psO,
                tc.tile_pool(name="psD", bufs=2, space="PSUM") as psD,
            ):
                for panel in range(NPANEL):
                    q0 = panel * 512
                    es = []
                    for kt in range(KST):
                        ps = psS.tile([P, 512], F32, name="psS")
                        for ot in range(OT):
                            nc.tensor.matmul(
                                ps,
                                lhsT=KT[ot][:, kt * P:(kt + 1) * P],
                                rhs=QT[ot][:, q0:q0 + 512],
                                start=(ot == 0),
                                stop=(ot == OT - 1),
                            )
                        bt = bt_pool.tile([P, 512], BF16, name="bt")
                        nc.sync.dma_start(
                            out=bt, in_=biasT_d[kt * P:(kt + 1) * P, q0:q0 + 512])
                        stg = stg_pool.tile([P, 512], F32, name="stg")
                        nc.vector.tensor_add(stg, ps, bt)
                        e = es_pool.tile([P, 512], BF16, name="es")
                        nc.scalar.activation(
                            e, stg, mybir.ActivationFunctionType.Exp)
                        es.append(e)

                    for j in range(4):
                        po = psO.tile([P, D], F32, name="psO")
                        pd = psD.tile([P, 1], F32, name="psD")
                        for kt in range(KST):
                            w_sb = es[kt][:, j * P:(j + 1) * P]
                            for half in range(DH):
                                nc.tensor.matmul(
                                    po[:, half * 512:(half + 1) * 512],
                                    lhsT=w_sb,
                                    rhs=V[kt][:, half * 512:(half + 1) * 512],
                                    start=(kt == 0),
                                    stop=(kt == KST - 1),
                                )
                            nc.tensor.matmul(
                                pd, lhsT=w_sb, rhs=ones,
                                start=(kt == 0), stop=(kt == KST - 1),
                            )
                        rec = rc_pool.tile([P, 1], F32, name="rc")
                        nc.vector.reciprocal(rec, pd)
                        ob = ob_pool.tile([P, D], F32, name="ob")
                        row = q0 + j * P
                        for half in range(2):
                            hs = slice(half * D // 2, (half + 1) * D // 2)
                            nc.scalar.activation(
                                ob[:, hs], po[:, hs],
                                mybir.ActivationFunctionType.Copy,
                                scale=rec[:, 0:1],
                            )
                            nc.sync.dma_start(
                                out=out_d[row:row + P, hs], in_=ob[:, hs])

    _dedup_ldweights(nc)
    nc.compile()
    return nc


_NC_CACHE: dict = {}


def _get_nc(S: int, D: int) -> bass.Bass:
    key = (S, D)
    if key not in _NC_CACHE:
        _NC_CACHE[key] = build_attention_nc(S, D)
    return _NC_CACHE[key]


def kernel(x, Wq, Wk, Wv, rel_pos_emb, rel_pos) -> np.ndarray:
    x = np.asarray(x, dtype=np.float32)
    Wq = np.asarray(Wq, dtype=np.float32)
    Wk = np.asarray(Wk, dtype=np.float32)
    Wv = np.asarray(Wv, dtype=np.float32)
    rel_pos_emb = np.asarray(rel_pos_emb, dtype=np.float32)
    rel_pos = np.asarray(rel_pos)

    b, S, D = x.shape
    assert b == B

    # host prep: layout transforms + bias table lookup
    scale = 1.0 / np.sqrt(np.float32(D))
    wqT = np.ascontiguousarray((Wq.T * scale)).astype(BF16_NP)
    wkT = np.ascontiguousarray(Wk.T).astype(BF16_NP)
    wvT = np.ascontiguousarray(Wv.T).astype(BF16_NP)
    bias = rel_pos_emb[rel_pos[:S, :S], 0]          # [qs, ks]
    biasT = np.ascontiguousarray(bias.T).astype(BF16_NP)  # [ks, qs]

    in_maps = []
    for i in range(N_CORES):
        in_maps.append({
            "xT": np.ascontiguousarray(x[i].T).astype(BF16_NP),
            "wqT": wqT,
            "wkT": wkT,
            "wvT": wvT,
            "biasT": biasT,
        })

    nc = _get_nc(S, D)
    res = run_bass_kernel_spmd(
        nc, in_maps, core_ids=list(range(N_CORES)), **_RUN_KWARGS)
    global LAST_RESULT
    LAST_RESULT = res
    return np.stack([r["out"] for r in res.results]).astype(np.float32)


# test harness hooks: set _RUN_KWARGS = {"trace": True} before calling kernel()
# to capture the NTFF profile; the full BassKernelResults lands in LAST_RESULT.
_RUN_KWARGS: dict = {}
LAST_RESULT = None
